# revision 2
# baseline (speedup 1.0000x reference)
"""Trainium2 Bass kernel for the AlignmentModule problem.

Data-parallel over batch: 32 batch elements -> 4 per NeuronCore x 8 cores.
Per batch element:
  text branch: conv(k3,relu) -> conv(k1)            (channels-major layout)
  feats branch: conv(k3,relu) x2 -> conv(k1)
  pairwise dist^2 = f2 + t2 - 2 f.t  built in PSUM via matmul accumulation
  out = -sqrt(dist^2) - log(sum_j exp(-sqrt(dist^2)))   (log-softmax, no
  max-shift needed since scores <= 0), masked cols driven to -inf by adding
  +inf to t2 row before the sqrt.

All matmuls in bf16 (f32 PSUM accumulation). The -2 factor is folded into
the text conv2 weights host-side, so the ft matmul directly accumulates
-2*f.t and t2 is recovered from (-2t)^2 * 0.25.
"""

import numpy as np
import ml_dtypes

BF16 = ml_dtypes.bfloat16

ADIM, ODIM = 256, 80
B, T, F = 32, 500, 2000
NCORES = 8
BL = B // NCORES  # batches per core
CH = 128
NCC = ADIM // CH  # 2 channel chunks
NTT = F // T  # 4 feats time tiles
ICHUNKS = [(i * CH, min(CH, F - i * CH)) for i in range((F + CH - 1) // CH)]

_CACHE = {}


def _build(bl=BL):
    import concourse.bacc as bacc
    import concourse.mybir as mybir
    import concourse.tile as tile
    from concourse.tile_rust import add_dep_helper

    dt = mybir.dt
    AF = mybir.ActivationFunctionType
    ALU = mybir.AluOpType

    nc = bacc.Bacc(None, target_bir_lowering=False)

    textt = nc.dram_tensor("textt", [bl, NCC, CH, T], dt.bfloat16, kind="ExternalInput")
    featst = nc.dram_tensor("featst", [bl, ODIM, F], dt.bfloat16, kind="ExternalInput")
    maskv = nc.dram_tensor("maskv", [bl, T], dt.float32, kind="ExternalInput")
    tw1d = nc.dram_tensor("tw1", [CH, 6 * ADIM], dt.bfloat16, kind="ExternalInput")
    tw2d = nc.dram_tensor("tw2", [CH, 2 * ADIM], dt.bfloat16, kind="ExternalInput")
    fw1d = nc.dram_tensor("fw1", [ODIM, 3 * ADIM], dt.bfloat16, kind="ExternalInput")
    fw2d = nc.dram_tensor("fw2", [CH, 6 * ADIM], dt.bfloat16, kind="ExternalInput")
    fw3d = nc.dram_tensor("fw3", [CH, 2 * ADIM], dt.bfloat16, kind="ExternalInput")
    biasd = nc.dram_tensor("biases", [CH, 10], dt.float32, kind="ExternalInput")
    outd = nc.dram_tensor("out", [bl, F, T], dt.float32, kind="ExternalOutput")

    with tile.TileContext(nc) as tc:
        with (
            tc.tile_pool(name="consts", bufs=1) as cpool,
            tc.tile_pool(name="wpool", bufs=1) as wpool,
            tc.tile_pool(name="inp", bufs=2) as inpool,
            tc.tile_pool(name="acts", bufs=2) as apool,
            tc.tile_pool(name="fbuf", bufs=2) as fpool,
            tc.tile_pool(name="sq", bufs=3) as sqpool,
            tc.tile_pool(name="rows", bufs=2) as rpool,
            tc.tile_pool(name="dist", bufs=18) as dpool,
            tc.tile_pool(name="escr", bufs=2) as epool,
            tc.tile_pool(name="sums", bufs=2) as spool,
            tc.tile_pool(name="outs", bufs=4) as opool,
            tc.tile_pool(name="psconv", bufs=3, space="PSUM") as psconv,
            tc.tile_pool(name="psft", bufs=3, space="PSUM") as psft,
            tc.tile_pool(name="psrow", bufs=2, space="PSUM") as psrow,
        ):
            # constants
            ones_row = cpool.tile([1, 512], dt.bfloat16)
            nc.vector.memset(ones_row[:], 1.0)
            ones_col = cpool.tile([CH, 1], dt.bfloat16)
            nc.vector.memset(ones_col[:], 1.0)
            quarter_col = cpool.tile([CH, 1], dt.bfloat16)
            nc.vector.memset(quarter_col[:], 0.25)

            # weights / biases
            tw1 = wpool.tile([CH, 6 * ADIM], dt.bfloat16)
            nc.sync.dma_start(tw1[:], tw1d[:])
            tw2 = wpool.tile([CH, 2 * ADIM], dt.bfloat16)
            nc.sync.dma_start(tw2[:], tw2d[:])
            fw1 = wpool.tile([ODIM, 3 * ADIM], dt.bfloat16)
            nc.sync.dma_start(fw1[:], fw1d[:])
            fw2 = wpool.tile([CH, 6 * ADIM], dt.bfloat16)
            nc.sync.dma_start(fw2[:], fw2d[:])
            fw3 = wpool.tile([CH, 2 * ADIM], dt.bfloat16)
            nc.sync.dma_start(fw3[:], fw3d[:])
            bias = wpool.tile([CH, 10], dt.float32)
            nc.sync.dma_start(bias[:], biasd[:])

            # explicit ordering chain for the table-set-sensitive ACT ops
            act_chain = []

            def chain(inst):
                if act_chain:
                    add_dep_helper(inst.ins, act_chain[-1].ins, sync=False)
                act_chain.append(inst)

            for b in range(bl):
                # ---------------- text branch ----------------
                xts = []
                for c in range(NCC):
                    xt = inpool.tile([CH, T + 2], dt.bfloat16, tag=f"xt{c}")
                    nc.gpsimd.memset(xt[:, 0:1], 0.0)
                    nc.gpsimd.memset(xt[:, T + 1 : T + 2], 0.0)
                    nc.sync.dma_start(xt[:, 1 : T + 1], textt[b, c])
                    xts.append(xt)
                ths = []
                for co in range(NCC):
                    ps = psconv.tile([CH, T], dt.float32, tag="psc")
                    idx = 0
                    for ci in range(NCC):
                        for k in range(3):
                            s = (ci * 3 + k) * ADIM + co * CH
                            nc.tensor.matmul(
                                ps[:], tw1[:, s : s + CH], xts[ci][:, k : k + T],
                                start=(idx == 0), stop=(idx == 5),
                            )
                            idx += 1
                    th = apool.tile([CH, T], dt.bfloat16, tag=f"th{co}")
                    nc.scalar.activation(th[:], ps[:], AF.Relu, bias=bias[:, co : co + 1], scale=1.0)
                    ths.append(th)
                # conv2 (k=1), weights pre-scaled by -2 -> tp = -2*t
                tps = []
                for co in range(NCC):
                    ps = psconv.tile([CH, T], dt.float32, tag="psc")
                    for ci in range(NCC):
                        s = ci * ADIM + co * CH
                        nc.tensor.matmul(
                            ps[:], tw2[:, s : s + CH], ths[ci][:],
                            start=(ci == 0), stop=(ci == 1),
                        )
                    tp = apool.tile([CH, T], dt.bfloat16, tag=f"tp{co}")
                    nc.scalar.activation(tp[:], ps[:], AF.Identity, bias=bias[:, 2 + co : 3 + co], scale=1.0)
                    tps.append(tp)
                # t2 row (+ mask): t2 = 0.25 * sum_c tp^2 ; t2m = t2 + maskinf
                pst2 = psrow.tile([1, T], dt.float32, tag="psr")
                for c in range(NCC):
                    tsq = sqpool.tile([CH, T], dt.bfloat16, tag="sq")
                    nc.vector.tensor_mul(tsq[:], tps[c][:], tps[c][:])
                    nc.tensor.matmul(pst2[:], quarter_col[:], tsq[:], start=(c == 0), stop=(c == 1))
                masks = rpool.tile([1, T], dt.float32, tag="mask")
                nc.sync.dma_start(masks[:], maskv[b : b + 1, :])
                t2m = rpool.tile([1, T], dt.bfloat16, tag="t2m")
                nc.vector.tensor_add(t2m[:], pst2[:], masks[:])

                # ---------------- feats branch ----------------
                xf = inpool.tile([ODIM, F + 2], dt.bfloat16, tag="xf")
                nc.gpsimd.memset(xf[:, 0:1], 0.0)
                nc.gpsimd.memset(xf[:, F + 1 : F + 2], 0.0)
                nc.sync.dma_start(xf[:, 1 : F + 1], featst[b])
                h1s = []
                for c in range(NCC):
                    h1 = apool.tile([CH, F + 2], dt.bfloat16, tag=f"h1f{c}")
                    nc.gpsimd.memset(h1[:, 0:1], 0.0)
                    nc.gpsimd.memset(h1[:, F + 1 : F + 2], 0.0)
                    h1s.append(h1)
                for co in range(NCC):
                    for tt in range(NTT):
                        ps = psconv.tile([CH, T], dt.float32, tag="psc")
                        for k in range(3):
                            s = k * ADIM + co * CH
                            nc.tensor.matmul(
                                ps[:], fw1[:, s : s + CH],
                                xf[:, tt * T + k : tt * T + k + T],
                                start=(k == 0), stop=(k == 2),
                            )
                        nc.vector.tensor_scalar(
                            h1s[co][:, 1 + tt * T : 1 + (tt + 1) * T], ps[:],
                            bias[:, 4 + co : 5 + co], 0.0, ALU.add, ALU.max,
                        )
                h2s = []
                for c in range(NCC):
                    h2 = apool.tile([CH, F + 2], dt.bfloat16, tag=f"h2f{c}")
                    nc.gpsimd.memset(h2[:, 0:1], 0.0)
                    nc.gpsimd.memset(h2[:, F + 1 : F + 2], 0.0)
                    h2s.append(h2)
                for co in range(NCC):
                    for tt in range(NTT):
                        ps = psconv.tile([CH, T], dt.float32, tag="psc")
                        idx = 0
                        for ci in range(NCC):
                            for k in range(3):
                                s = (ci * 3 + k) * ADIM + co * CH
                                nc.tensor.matmul(
                                    ps[:], fw2[:, s : s + CH],
                                    h1s[ci][:, tt * T + k : tt * T + k + T],
                                    start=(idx == 0), stop=(idx == 5),
                                )
                                idx += 1
                        nc.scalar.activation(
                            h2s[co][:, 1 + tt * T : 1 + (tt + 1) * T], ps[:],
                            AF.Relu, bias=bias[:, 6 + co : 7 + co], scale=1.0,
                        )
                fs = []
                for c in range(NCC):
                    f_ = fpool.tile([CH, F], dt.bfloat16, tag=f"f{c}")
                    fs.append(f_)
                for co in range(NCC):
                    for tt in range(NTT):
                        ps = psconv.tile([CH, T], dt.float32, tag="psc")
                        for ci in range(NCC):
                            s = ci * ADIM + co * CH
                            nc.tensor.matmul(
                                ps[:], fw3[:, s : s + CH],
                                h2s[ci][:, 1 + tt * T : 1 + (tt + 1) * T],
                                start=(ci == 0), stop=(ci == 1),
                            )
                        nc.vector.tensor_scalar_add(
                            fs[co][:, tt * T : (tt + 1) * T], ps[:], bias[:, 8 + co : 9 + co]
                        )
                # f2 row: sum_c f^2 as a [1, F] row
                f2row = rpool.tile([1, F], dt.bfloat16, tag="f2row")
                for tt in range(NTT):
                    psf2 = psrow.tile([1, T], dt.float32, tag="psr")
                    for c in range(NCC):
                        fsq = sqpool.tile([CH, T], dt.bfloat16, tag="sq")
                        nc.vector.tensor_mul(
                            fsq[:], fs[c][:, tt * T : (tt + 1) * T], fs[c][:, tt * T : (tt + 1) * T]
                        )
                        nc.tensor.matmul(psf2[:], ones_col[:], fsq[:], start=(c == 0), stop=(c == 1))
                    nc.vector.tensor_copy(f2row[:, tt * T : (tt + 1) * T], psf2[:])

                # ---------------- distance + log-softmax ----------------
                s_all = spool.tile([CH, len(ICHUNKS)], dt.float32, tag="sall")
                nc.vector.memset(s_all[:], 1.0)
                dists = []
                for idx, (i0, m) in enumerate(ICHUNKS):
                    ps = psft.tile([CH, T], dt.float32, tag="psf")
                    nc.tensor.matmul(ps[:m], fs[0][:, i0 : i0 + m], tps[0][:], start=True, stop=False)
                    nc.tensor.matmul(ps[:m], fs[1][:, i0 : i0 + m], tps[1][:], start=False, stop=False)
                    nc.tensor.matmul(ps[:m], f2row[:, i0 : i0 + m], ones_row[:1, :T], start=False, stop=False)
                    nc.tensor.matmul(ps[:m], ones_row[:1, :m], t2m[:], start=False, stop=True)
                    d_ = dpool.tile([CH, T], dt.float32, tag="dist")
                    inst = nc.scalar.activation(d_[:m], ps[:m], AF.Sqrt)
                    chain(inst)
                    dists.append((d_, i0, m, idx))
                for d_, i0, m, idx in dists:
                    e_ = epool.tile([CH, T], dt.bfloat16, tag="e")
                    inst = nc.scalar.activation(
                        e_[:m], d_[:m], AF.Exp, scale=-1.0,
                        accum_out=s_all[:m, idx : idx + 1],
                    )
                    chain(inst)
                logs = spool.tile([CH, len(ICHUNKS)], dt.float32, tag="logs")
                inst = nc.scalar.activation(logs[:], s_all[:], AF.Ln)
                chain(inst)
                for d_, i0, m, idx in dists:
                    o_ = opool.tile([CH, T], dt.float32, tag="o")
                    nc.vector.tensor_scalar(
                        o_[:m], d_[:m], logs[:m, idx : idx + 1], -1.0, ALU.add, ALU.mult
                    )
                    nc.sync.dma_start(outd[b, i0 : i0 + m, :], o_[:m])

    nc.compile()
    return nc


def _get_nc(bl=BL):
    key = ("nc", bl)
    if key not in _CACHE:
        _CACHE[key] = _build(bl)
    return _CACHE[key]


def _prep(inputs):
    text = np.asarray(inputs["text"], np.float32)
    feats = np.asarray(inputs["feats"], np.float32)
    xm = np.asarray(inputs["x_masks"]).astype(bool)
    tW1 = np.asarray(inputs["tW1"], np.float32)
    tb1 = np.asarray(inputs["tb1"], np.float32)
    tW2 = np.asarray(inputs["tW2"], np.float32)
    tb2 = np.asarray(inputs["tb2"], np.float32)
    fW1 = np.asarray(inputs["fW1"], np.float32)
    fb1 = np.asarray(inputs["fb1"], np.float32)
    fW2 = np.asarray(inputs["fW2"], np.float32)
    fb2 = np.asarray(inputs["fb2"], np.float32)
    fW3 = np.asarray(inputs["fW3"], np.float32)
    fb3 = np.asarray(inputs["fb3"], np.float32)

    textt = np.ascontiguousarray(text.transpose(0, 2, 1)).reshape(B, NCC, CH, T).astype(BF16)
    featst = np.ascontiguousarray(feats.transpose(0, 2, 1)).astype(BF16)
    maskv = np.where(xm, np.inf, 0.0).astype(np.float32)

    def pack_k3(W):  # (co, 256, 3) -> (128, [cc][k][co])
        t = W.transpose(1, 2, 0).reshape(NCC, CH, 3, ADIM)
        return np.ascontiguousarray(t.transpose(1, 0, 2, 3).reshape(CH, 6 * ADIM)).astype(BF16)

    def pack_k1(W):  # (co, 256) -> (128, [cc][co])
        t = W.T.reshape(NCC, CH, ADIM)
        return np.ascontiguousarray(t.transpose(1, 0, 2).reshape(CH, 2 * ADIM)).astype(BF16)

    tw1 = pack_k3(tW1)
    tw2 = pack_k1(-2.0 * tW2[:, :, 0])
    fw1 = np.ascontiguousarray(fW1.transpose(1, 2, 0).reshape(ODIM, 3 * ADIM)).astype(BF16)
    fw2 = pack_k3(fW2)
    fw3 = pack_k1(fW3[:, :, 0])
    bias = np.zeros((CH, 10), np.float32)
    bias[:, 0:2] = tb1.reshape(NCC, CH).T
    bias[:, 2:4] = (-2.0 * tb2).reshape(NCC, CH).T
    bias[:, 4:6] = fb1.reshape(NCC, CH).T
    bias[:, 6:8] = fb2.reshape(NCC, CH).T
    bias[:, 8:10] = fb3.reshape(NCC, CH).T

    shared = {
        "tw1": tw1, "tw2": tw2, "fw1": fw1, "fw2": fw2, "fw3": fw3, "biases": bias,
    }
    in_maps = []
    for i in range(NCORES):
        m = dict(shared)
        m["textt"] = textt[i * BL : (i + 1) * BL]
        m["featst"] = featst[i * BL : (i + 1) * BL]
        m["maskv"] = maskv[i * BL : (i + 1) * BL]
        in_maps.append(m)
    return in_maps


def run(inputs, trace=False):
    from concourse.bass_utils import run_bass_kernel_spmd

    nc = _get_nc()
    in_maps = _prep(inputs)
    res = run_bass_kernel_spmd(nc, in_maps, core_ids=list(range(NCORES)), trace=trace)
    out = np.concatenate([res.results[i]["out"] for i in range(NCORES)], axis=0)
    return out, res


def kernel(**inputs):
    out, _ = run(inputs, trace=False)
    return out


# revision 16
# speedup vs baseline: 1.1160x; 1.1160x over previous
"""Trainium2 Bass kernel for the AlignmentModule problem.

Data-parallel over batch: 32 batch elements -> 4 per NeuronCore x 8 cores.
Per batch element:
  text branch: conv(k3,relu) -> conv(k1)            (channels-major layout)
  feats branch: conv(k3,relu) x2 -> conv(k1)
  pairwise dist^2 = f2 + t2 - 2 f.t  built in PSUM via matmul accumulation
  out = -sqrt(dist^2) - log(sum_j exp(-sqrt(dist^2)))   (log-softmax, no
  max-shift needed since scores <= 0), masked cols driven to -inf by adding
  +inf to t2 row before the sqrt.

All matmuls in bf16 (f32 PSUM accumulation). The -2 factor is folded into
the text conv2 weights host-side, so the ft matmul directly accumulates
-2*f.t and t2 is recovered from (-2t)^2 * 0.25.
"""

import numpy as np
import ml_dtypes

BF16 = ml_dtypes.bfloat16

ADIM, ODIM = 256, 80
B, T, F = 32, 500, 2000
NCORES = 8
BL = B // NCORES  # batches per core
CH = 128
NCC = ADIM // CH  # 2 channel chunks
NTT = F // T  # 4 feats time tiles
ICHUNKS = [(i * CH, min(CH, F - i * CH)) for i in range((F + CH - 1) // CH)]

_CACHE = {}
_PATCH_ACT_TABLES = False


def _patch_act_tables():
    """Prefer natural_log_exp_and_others for Exp so the per-batch exp+ln
    phase costs one table load instead of two."""
    import concourse.bacc as bacc
    import concourse.hw_specs as hw_specs

    if getattr(bacc, "_act_tables_reordered", False):
        return
    orig = hw_specs.get_activation_tables

    def reordered(arch):
        t = orig(arch)
        pref = "natural_log_exp_and_others"
        if pref not in t:
            return t
        order = [pref] + [k for k in t if k != pref]
        return {k: t[k] for k in order}

    bacc.get_activation_tables = reordered
    bacc._act_tables_reordered = True


def _build(bl=BL):
    import concourse.bacc as bacc
    import concourse.mybir as mybir
    import concourse.tile as tile
    from concourse.tile_rust import add_dep_helper

    dt = mybir.dt
    AF = mybir.ActivationFunctionType
    ALU = mybir.AluOpType

    if _PATCH_ACT_TABLES:
        _patch_act_tables()
    nc = bacc.Bacc(None, target_bir_lowering=False)

    textt = nc.dram_tensor("textt", [bl, NCC, CH, T], dt.bfloat16, kind="ExternalInput")
    featst = nc.dram_tensor("featst", [bl, ODIM, F], dt.bfloat16, kind="ExternalInput")
    maskv = nc.dram_tensor("maskv", [bl, T], dt.float32, kind="ExternalInput")
    tw1d = nc.dram_tensor("tw1", [CH, 6 * ADIM], dt.bfloat16, kind="ExternalInput")
    tw2d = nc.dram_tensor("tw2", [CH, 2 * ADIM], dt.bfloat16, kind="ExternalInput")
    fw1d = nc.dram_tensor("fw1", [ODIM, 3 * ADIM], dt.bfloat16, kind="ExternalInput")
    fw2d = nc.dram_tensor("fw2", [CH, 6 * ADIM], dt.bfloat16, kind="ExternalInput")
    fw3d = nc.dram_tensor("fw3", [CH, 2 * ADIM], dt.bfloat16, kind="ExternalInput")
    biasd = nc.dram_tensor("biases", [CH, 10], dt.float32, kind="ExternalInput")
    outd = nc.dram_tensor("out", [bl, F, T], dt.float32, kind="ExternalOutput")

    with tile.TileContext(nc) as tc:
        with (
            tc.tile_pool(name="consts", bufs=1) as cpool,
            tc.tile_pool(name="wpool", bufs=1) as wpool,
            tc.tile_pool(name="inp", bufs=2) as inpool,
            tc.tile_pool(name="acts", bufs=2) as apool,
            tc.tile_pool(name="fbuf", bufs=2) as fpool,
            tc.tile_pool(name="sq", bufs=3) as sqpool,
            tc.tile_pool(name="rows", bufs=2) as rpool,
            tc.tile_pool(name="dist", bufs=18) as dpool,
            tc.tile_pool(name="escr", bufs=2) as epool,
            tc.tile_pool(name="sums", bufs=2) as spool,
            tc.tile_pool(name="outs", bufs=4) as opool,
            tc.tile_pool(name="psconv", bufs=4, space="PSUM") as psconv,
            tc.tile_pool(name="psft", bufs=2, space="PSUM") as psft,
            tc.tile_pool(name="psrow", bufs=2, space="PSUM") as psrow,
        ):
            # constants
            ones_rowT = cpool.tile([1, T], dt.bfloat16)
            nc.vector.memset(ones_rowT[:], 1.0)
            # [1, 0] / [0, 1] selector rows and [f2|0] / [0|t2] column weights
            # for building the 2-row augmented operands in PSUM
            sel10 = cpool.tile([1, 2], dt.bfloat16)
            nc.vector.memset(sel10[:, 0:1], 1.0)
            nc.vector.memset(sel10[:, 1:2], 0.0)
            sel01 = cpool.tile([1, 2], dt.bfloat16)
            nc.vector.memset(sel01[:, 0:1], 0.0)
            nc.vector.memset(sel01[:, 1:2], 1.0)
            onescol2 = cpool.tile([CH, 2], dt.bfloat16)  # [1, 0] per partition
            nc.vector.memset(onescol2[:, 0:1], 1.0)
            nc.vector.memset(onescol2[:, 1:2], 0.0)
            qcol2 = cpool.tile([CH, 2], dt.bfloat16)  # [0, 0.25] per partition
            nc.vector.memset(qcol2[:, 0:1], 0.0)
            nc.vector.memset(qcol2[:, 1:2], 0.25)

            # weights / biases
            tw1 = wpool.tile([CH, 6 * ADIM], dt.bfloat16)
            nc.sync.dma_start(tw1[:], tw1d[:])
            tw2 = wpool.tile([CH, 2 * ADIM], dt.bfloat16)
            nc.sync.dma_start(tw2[:], tw2d[:])
            fw1 = wpool.tile([ODIM, 3 * ADIM], dt.bfloat16)
            nc.sync.dma_start(fw1[:], fw1d[:])
            fw2 = wpool.tile([CH, 6 * ADIM], dt.bfloat16)
            nc.sync.dma_start(fw2[:], fw2d[:])
            fw3 = wpool.tile([CH, 2 * ADIM], dt.bfloat16)
            nc.sync.dma_start(fw3[:], fw3d[:])
            bias = wpool.tile([CH, 10], dt.float32)
            nc.sync.dma_start(bias[:], biasd[:])

            # explicit ordering chain for the table-set-sensitive ACT ops
            act_chain = []

            def chain(inst):
                if act_chain:
                    add_dep_helper(inst.ins, act_chain[-1].ins, sync=False)
                act_chain.append(inst)

            for b in range(bl):
                # ---------------- text branch ----------------
                xts = []
                for c in range(NCC):
                    xt = inpool.tile([CH, T + 2], dt.bfloat16, tag=f"xt{c}")
                    nc.gpsimd.memset(xt[:, 0:1], 0.0)
                    nc.gpsimd.memset(xt[:, T + 1 : T + 2], 0.0)
                    nc.sync.dma_start(xt[:, 1 : T + 1], textt[b, c])
                    xts.append(xt)
                ths = []
                for co in range(NCC):
                    ps = psconv.tile([CH, T], dt.float32, tag="psc")
                    idx = 0
                    for ci in range(NCC):
                        for k in range(3):
                            s = (ci * 3 + k) * ADIM + co * CH
                            nc.tensor.matmul(
                                ps[:], tw1[:, s : s + CH], xts[ci][:, k : k + T],
                                start=(idx == 0), stop=(idx == 5),
                            )
                            idx += 1
                    th = apool.tile([CH, T], dt.bfloat16, tag=f"th{co}")
                    nc.scalar.activation(th[:], ps[:], AF.Relu, bias=bias[:, co : co + 1], scale=1.0)
                    ths.append(th)
                # conv2 (k=1), weights pre-scaled by -2 -> tp = -2*t
                tps = []
                for co in range(NCC):
                    ps = psconv.tile([CH, T], dt.float32, tag="psc")
                    for ci in range(NCC):
                        s = ci * ADIM + co * CH
                        nc.tensor.matmul(
                            ps[:], tw2[:, s : s + CH], ths[ci][:],
                            start=(ci == 0), stop=(ci == 1),
                        )
                    tp = apool.tile([CH, T], dt.bfloat16, tag=f"tp{co}")
                    nc.scalar.activation(tp[:], ps[:], AF.Identity, bias=bias[:, 2 + co : 3 + co], scale=1.0)
                    tps.append(tp)
                # aug_r = [[ones], [t2 + maskinf]] built in PSUM:
                #   row1 = 0.25 * sum_c tp^2 (qcol2), row0 = 1 (sel10 x ones)
                pst2 = psrow.tile([2, T], dt.float32, tag="psr")
                for c in range(NCC):
                    tsq = sqpool.tile([CH, T], dt.bfloat16, tag="sq")
                    nc.vector.tensor_mul(tsq[:], tps[c][:], tps[c][:])
                    nc.tensor.matmul(pst2[:], qcol2[:], tsq[:], start=(c == 0), stop=False)
                nc.tensor.matmul(pst2[:], sel10[:], ones_rowT[:], start=False, stop=True)
                masks = rpool.tile([2, T], dt.float32, tag="mask")
                nc.vector.memset(masks[0:1, :], 0.0)
                nc.sync.dma_start(masks[1:2, :], maskv[b : b + 1, :])
                aug_r = rpool.tile([2, T], dt.bfloat16, tag="augr")
                nc.vector.tensor_add(aug_r[:], pst2[:], masks[:])

                # ---------------- feats branch ----------------
                xf = inpool.tile([ODIM, F + 2], dt.bfloat16, tag="xf")
                nc.gpsimd.memset(xf[:, 0:1], 0.0)
                nc.gpsimd.memset(xf[:, F + 1 : F + 2], 0.0)
                nc.sync.dma_start(xf[:, 1 : F + 1], featst[b])
                h1s = []
                for c in range(NCC):
                    h1 = apool.tile([CH, F + 2], dt.bfloat16, tag=f"h1f{c}")
                    nc.gpsimd.memset(h1[:, 0:1], 0.0)
                    nc.gpsimd.memset(h1[:, F + 1 : F + 2], 0.0)
                    h1s.append(h1)
                # conv1f: one LDWEIGHTS per (co,k), 4 time-tiles accumulate in
                # 4 psum banks concurrently
                for co in range(NCC):
                    pss = [psconv.tile([CH, T], dt.float32, tag="psc", name=f"psc_{b}_{co}_{i}") for i in range(NTT)]
                    for k in range(3):
                        s = k * ADIM + co * CH
                        for tt in range(NTT):
                            nc.tensor.matmul(
                                pss[tt][:], fw1[:, s : s + CH],
                                xf[:, tt * T + k : tt * T + k + T],
                                start=(k == 0), stop=(k == 2),
                            )
                    for tt in range(NTT):
                        nc.vector.tensor_scalar(
                            h1s[co][:, 1 + tt * T : 1 + (tt + 1) * T], pss[tt][:],
                            bias[:, 4 + co : 5 + co], 0.0, ALU.add, ALU.max,
                        )
                h2s = []
                for c in range(NCC):
                    h2 = apool.tile([CH, F + 2], dt.bfloat16, tag=f"h2f{c}")
                    nc.gpsimd.memset(h2[:, 0:1], 0.0)
                    nc.gpsimd.memset(h2[:, F + 1 : F + 2], 0.0)
                    h2s.append(h2)
                # conv2f: one LDWEIGHTS per (co,ci,k), 4 time-tiles in flight
                for co in range(NCC):
                    pss = [psconv.tile([CH, T], dt.float32, tag="psc", name=f"psc_{b}_{co}_{i}") for i in range(NTT)]
                    idx = 0
                    for ci in range(NCC):
                        for k in range(3):
                            s = (ci * 3 + k) * ADIM + co * CH
                            for tt in range(NTT):
                                nc.tensor.matmul(
                                    pss[tt][:], fw2[:, s : s + CH],
                                    h1s[ci][:, tt * T + k : tt * T + k + T],
                                    start=(idx == 0), stop=(idx == 5),
                                )
                            idx += 1
                    for tt in range(NTT):
                        nc.scalar.activation(
                            h2s[co][:, 1 + tt * T : 1 + (tt + 1) * T], pss[tt][:],
                            AF.Relu, bias=bias[:, 6 + co : 7 + co], scale=1.0,
                        )
                fs = []
                for c in range(NCC):
                    f_ = fpool.tile([CH, F], dt.bfloat16, tag=f"f{c}")
                    fs.append(f_)
                # conv3f: one LDWEIGHTS per (co,ci), 4 time-tiles in flight
                for co in range(NCC):
                    pss = [psconv.tile([CH, T], dt.float32, tag="psc", name=f"psc_{b}_{co}_{i}") for i in range(NTT)]
                    for ci in range(NCC):
                        s = ci * ADIM + co * CH
                        for tt in range(NTT):
                            nc.tensor.matmul(
                                pss[tt][:], fw3[:, s : s + CH],
                                h2s[ci][:, 1 + tt * T : 1 + (tt + 1) * T],
                                start=(ci == 0), stop=(ci == 1),
                            )
                    for tt in range(NTT):
                        nc.vector.tensor_scalar_add(
                            fs[co][:, tt * T : (tt + 1) * T], pss[tt][:], bias[:, 8 + co : 9 + co]
                        )
                # aug_l = [[f2row], [ones]] built in PSUM per time-tile:
                #   row0 = sum_c f^2 (onescol2), row1 = 1 (sel01 x ones)
                aug_l = rpool.tile([2, F], dt.bfloat16, tag="augl")
                for tt in range(NTT):
                    psf2 = psrow.tile([2, T], dt.float32, tag="psr")
                    for c in range(NCC):
                        fsq = sqpool.tile([CH, T], dt.bfloat16, tag="sq")
                        nc.vector.tensor_mul(
                            fsq[:], fs[c][:, tt * T : (tt + 1) * T], fs[c][:, tt * T : (tt + 1) * T]
                        )
                        nc.tensor.matmul(psf2[:], onescol2[:], fsq[:], start=(c == 0), stop=False)
                    nc.tensor.matmul(psf2[:], sel01[:], ones_rowT[:], start=False, stop=True)
                    nc.vector.tensor_copy(aug_l[:, tt * T : (tt + 1) * T], psf2[:])

                # ---------------- distance + log-softmax ----------------
                s_all = spool.tile([CH, len(ICHUNKS)], dt.float32, tag="sall")
                nc.vector.memset(s_all[:], 1.0)
                dists = []
                for idx, (i0, m) in enumerate(ICHUNKS):
                    ps = psft.tile([CH, T], dt.float32, tag="psf")
                    nc.tensor.matmul(ps[:m], fs[0][:, i0 : i0 + m], tps[0][:], start=True, stop=False)
                    nc.tensor.matmul(ps[:m], fs[1][:, i0 : i0 + m], tps[1][:], start=False, stop=False)
                    nc.tensor.matmul(ps[:m], aug_l[:, i0 : i0 + m], aug_r[:, :T], start=False, stop=True)
                    d_ = dpool.tile([CH, T], dt.float32, tag="dist")
                    inst = nc.scalar.activation(d_[:m], ps[:m], AF.Sqrt)
                    chain(inst)
                    dists.append((d_, i0, m, idx))
                for d_, i0, m, idx in dists:
                    e_ = epool.tile([CH, T], dt.bfloat16, tag="e")
                    inst = nc.scalar.activation(
                        e_[:m], d_[:m], AF.Exp, scale=-1.0,
                        accum_out=s_all[:m, idx : idx + 1],
                    )
                    chain(inst)
                logs = spool.tile([CH, len(ICHUNKS)], dt.float32, tag="logs")
                inst = nc.scalar.activation(logs[:], s_all[:], AF.Ln)
                chain(inst)
                for d_, i0, m, idx in dists:
                    o_ = opool.tile([CH, T], dt.float32, tag="o")
                    nc.vector.tensor_scalar(
                        o_[:m], d_[:m], logs[:m, idx : idx + 1], -1.0, ALU.add, ALU.mult
                    )
                    nc.sync.dma_start(outd[b, i0 : i0 + m, :], o_[:m])

    nc.compile()
    return nc


def _get_nc(bl=BL):
    key = ("nc", bl)
    if key not in _CACHE:
        _CACHE[key] = _build(bl)
    return _CACHE[key]


def _prep(inputs):
    text = np.asarray(inputs["text"], np.float32)
    feats = np.asarray(inputs["feats"], np.float32)
    xm = np.asarray(inputs["x_masks"]).astype(bool)
    tW1 = np.asarray(inputs["tW1"], np.float32)
    tb1 = np.asarray(inputs["tb1"], np.float32)
    tW2 = np.asarray(inputs["tW2"], np.float32)
    tb2 = np.asarray(inputs["tb2"], np.float32)
    fW1 = np.asarray(inputs["fW1"], np.float32)
    fb1 = np.asarray(inputs["fb1"], np.float32)
    fW2 = np.asarray(inputs["fW2"], np.float32)
    fb2 = np.asarray(inputs["fb2"], np.float32)
    fW3 = np.asarray(inputs["fW3"], np.float32)
    fb3 = np.asarray(inputs["fb3"], np.float32)

    textt = np.ascontiguousarray(text.transpose(0, 2, 1)).reshape(B, NCC, CH, T).astype(BF16)
    featst = np.ascontiguousarray(feats.transpose(0, 2, 1)).astype(BF16)
    maskv = np.where(xm, np.inf, 0.0).astype(np.float32)

    def pack_k3(W):  # (co, 256, 3) -> (128, [cc][k][co])
        t = W.transpose(1, 2, 0).reshape(NCC, CH, 3, ADIM)
        return np.ascontiguousarray(t.transpose(1, 0, 2, 3).reshape(CH, 6 * ADIM)).astype(BF16)

    def pack_k1(W):  # (co, 256) -> (128, [cc][co])
        t = W.T.reshape(NCC, CH, ADIM)
        return np.ascontiguousarray(t.transpose(1, 0, 2).reshape(CH, 2 * ADIM)).astype(BF16)

    tw1 = pack_k3(tW1)
    tw2 = pack_k1(-2.0 * tW2[:, :, 0])
    fw1 = np.ascontiguousarray(fW1.transpose(1, 2, 0).reshape(ODIM, 3 * ADIM)).astype(BF16)
    fw2 = pack_k3(fW2)
    fw3 = pack_k1(fW3[:, :, 0])
    bias = np.zeros((CH, 10), np.float32)
    bias[:, 0:2] = tb1.reshape(NCC, CH).T
    bias[:, 2:4] = (-2.0 * tb2).reshape(NCC, CH).T
    bias[:, 4:6] = fb1.reshape(NCC, CH).T
    bias[:, 6:8] = fb2.reshape(NCC, CH).T
    bias[:, 8:10] = fb3.reshape(NCC, CH).T

    shared = {
        "tw1": tw1, "tw2": tw2, "fw1": fw1, "fw2": fw2, "fw3": fw3, "biases": bias,
    }
    in_maps = []
    for i in range(NCORES):
        m = dict(shared)
        m["textt"] = textt[i * BL : (i + 1) * BL]
        m["featst"] = featst[i * BL : (i + 1) * BL]
        m["maskv"] = maskv[i * BL : (i + 1) * BL]
        in_maps.append(m)
    return in_maps


def run(inputs, trace=False):
    from concourse.bass_utils import run_bass_kernel_spmd

    nc = _get_nc()
    in_maps = _prep(inputs)
    res = run_bass_kernel_spmd(nc, in_maps, core_ids=list(range(NCORES)), trace=trace)
    out = np.concatenate([res.results[i]["out"] for i in range(NCORES)], axis=0)
    return out, res


def kernel(**inputs):
    out, _ = run(inputs, trace=False)
    return out


# revision 39
# speedup vs baseline: 1.2711x; 1.1390x over previous
"""Trainium2 Bass kernel for the AlignmentModule problem.

Data-parallel over batch: 32 batch elements -> 4 per NeuronCore x 8 cores.
Per batch element:
  text branch: conv(k3,relu) -> conv(k1)            (channels-major layout)
  feats branch: conv(k3,relu) x2 -> conv(k1)
  pairwise dist^2 = f2 + t2 - 2 f.t  built in PSUM via matmul accumulation
  out = -sqrt(dist^2) - log(sum_j exp(-sqrt(dist^2)))   (log-softmax, no
  max-shift needed since scores <= 0), masked cols driven to -inf by adding
  +inf to t2 row before the sqrt.

All matmuls in bf16 (f32 PSUM accumulation). The -2 factor is folded into
the text conv2 weights host-side, so the ft matmul directly accumulates
-2*f.t and t2 is recovered from (-2t)^2 * 0.25.
"""

import numpy as np
import ml_dtypes

BF16 = ml_dtypes.bfloat16

ADIM, ODIM = 256, 80
B, T, F = 32, 500, 2000
NCORES = 8
BL = B // NCORES  # batches per core
CH = 128
NCC = ADIM // CH  # 2 channel chunks
NTT = F // T  # 4 feats time tiles
ICHUNKS = [(i * CH, min(CH, F - i * CH)) for i in range((F + CH - 1) // CH)]

_CACHE = {}
_PATCH_ACT_TABLES = False


def _patch_act_tables():
    """Prefer natural_log_exp_and_others for Exp so the per-batch exp+ln
    phase costs one table load instead of two."""
    import concourse.bacc as bacc
    import concourse.hw_specs as hw_specs

    if getattr(bacc, "_act_tables_reordered", False):
        return
    orig = hw_specs.get_activation_tables

    def reordered(arch):
        t = orig(arch)
        pref = "natural_log_exp_and_others"
        if pref not in t:
            return t
        order = [pref] + [k for k in t if k != pref]
        return {k: t[k] for k in order}

    bacc.get_activation_tables = reordered
    bacc._act_tables_reordered = True


def _dedupe_ldweights(nc):
    """Drop back-to-back InstLdweights that reload identical weights.

    bass emits one Ldweights per matmul; for weight-reuse loops (same
    stationary operand across several matmuls) the repeats are redundant.
    Only removes an Ldweights whose key (memref/ap/offset/dtype/mode)
    matches the immediately preceding weight load with no semaphore
    wait/update attached, so synchronization is preserved.
    """

    def key(inst):
        try:
            ap = inst.ins[0]
            k = (
                getattr(ap, "memref", None), str(getattr(ap, "ap", None)),
                getattr(ap, "offset", None), str(getattr(ap, "dtype", None)),
                str(inst.perf_mode), str(inst.is_transpose),
                str(inst.tile_position), str(inst.tile_size),
            )
            if k[0] is None:
                return None
            return k
        except Exception:
            return None

    removed = 0
    for bb in nc.m.functions[0].blocks:
        insts = list(bb.instructions)
        out = []
        last = None
        changed = False
        for inst in insts:
            tn = type(inst).__name__
            if tn == "InstLdweights":
                k = key(inst)
                if (
                    k is not None and k == last
                    and not inst.has_wait() and not inst.has_update()
                ):
                    removed += 1
                    changed = True
                    continue
                last = k
            elif tn != "InstMatmult":
                # non-PE instructions in the same block don't touch the PE
                # array; matmuls don't either. Anything else: keep the key.
                pass
            out.append(inst)
        if changed:
            bb.instructions = out
    return removed


def _retarget_act_table_loads(nc):
    """Post-compile: make Exp use the natural_log_exp set (it contains exp
    AND ln) and drop now-redundant consecutive same-set loads. Keeps the
    canonical set-id numbering, so walrus's name/id mapping is untouched."""
    from concourse.hw_specs import get_activation_tables

    try:
        tables = list(get_activation_tables(nc.m.arch).keys())
    except Exception:
        return 0
    try:
        nle_id = tables.index("natural_log_exp_and_others")
        remap = {
            tables.index("exp_and_others"): nle_id,
            tables.index("natural_log"): nle_id,
        }
    except ValueError:
        return 0
    removed = 0
    for bb in nc.m.functions[0].blocks:
        insts = list(bb.instructions)
        out = []
        cur = None
        changed = False
        for inst in insts:
            if type(inst).__name__ == "InstLoadActFuncSet":
                if inst.act_func_set_id in remap:
                    inst.act_func_set_id = remap[inst.act_func_set_id]
                    changed = True
                if (
                    inst.act_func_set_id == cur
                    and not inst.has_wait() and not inst.has_update()
                ):
                    removed += 1
                    changed = True
                    continue
                cur = inst.act_func_set_id
            out.append(inst)
        if changed:
            bb.instructions = out
    return removed


def _build(bl=BL):
    import concourse.bacc as bacc
    import concourse.mybir as mybir
    import concourse.tile as tile
    from concourse.tile_rust import add_dep_helper

    dt = mybir.dt
    AF = mybir.ActivationFunctionType
    ALU = mybir.AluOpType

    if _PATCH_ACT_TABLES:
        _patch_act_tables()
    nc = bacc.Bacc(None, target_bir_lowering=False)

    textt = nc.dram_tensor("textt", [bl, NCC, CH, T], dt.bfloat16, kind="ExternalInput")
    featst = nc.dram_tensor("featst", [bl, ODIM, F], dt.bfloat16, kind="ExternalInput")
    maskv = nc.dram_tensor("maskv", [bl, T], dt.float32, kind="ExternalInput")
    tw1d = nc.dram_tensor("tw1", [CH, 6 * ADIM], dt.bfloat16, kind="ExternalInput")
    tw2d = nc.dram_tensor("tw2", [CH, 2 * ADIM], dt.bfloat16, kind="ExternalInput")
    fw1ad = nc.dram_tensor("fw1a", [CH, ADIM], dt.bfloat16, kind="ExternalInput")
    fw1bd = nc.dram_tensor("fw1b", [112, ADIM], dt.bfloat16, kind="ExternalInput")
    fw2d = nc.dram_tensor("fw2", [CH, 2, 3 * ADIM], dt.float8e4, kind="ExternalInput")
    fw3d = nc.dram_tensor("fw3", [CH, 2, ADIM], dt.float8e4, kind="ExternalInput")
    biasd = nc.dram_tensor("biases", [CH, 10], dt.float32, kind="ExternalInput")
    onesd = nc.dram_tensor("onesd", [1, 512], dt.float32, kind="ExternalInput")
    outd = nc.dram_tensor("out", [bl, F, T], dt.float32, kind="ExternalOutput")

    with tile.TileContext(nc) as tc:
        with (
            tc.tile_pool(name="consts", bufs=1) as cpool,
            tc.tile_pool(name="wpool", bufs=1) as wpool,
            tc.tile_pool(name="inp", bufs=1) as inpool,
            tc.tile_pool(name="acts", bufs=2) as apool,
            tc.tile_pool(name="fbuf", bufs=2) as fpool,
            tc.tile_pool(name="sq", bufs=3) as sqpool,
            tc.tile_pool(name="rows", bufs=1) as rpool,
            tc.tile_pool(name="dist", bufs=18) as dpool,
            tc.tile_pool(name="escr", bufs=2) as epool,
            tc.tile_pool(name="sums", bufs=2) as spool,
            tc.tile_pool(name="outs", bufs=4) as opool,
            tc.tile_pool(name="psconv", bufs=4, space="PSUM") as psconv,
            tc.tile_pool(name="psft", bufs=2, space="PSUM") as psft,
            tc.tile_pool(name="psq", bufs=2, space="PSUM") as psq,
        ):
            # full-array "ones" weights: col0 of wf2 sums f^2 into psum row 0;
            # col1 of wt2 scales tp^2 by 0.25 into psum row 1
            wf2c = cpool.tile([CH, CH], dt.bfloat16)
            nc.vector.memset(wf2c[:], 0.0)
            nc.vector.memset(wf2c[:, 0:1], 1.0)
            wt2c = cpool.tile([CH, CH], dt.bfloat16)
            nc.vector.memset(wt2c[:], 0.0)
            nc.vector.memset(wt2c[:, 1:2], 0.25)

            # weights / biases
            tw1 = wpool.tile([CH, 6 * ADIM], dt.bfloat16)
            nc.sync.dma_start(tw1[:], tw1d[:])
            tw2 = wpool.tile([CH, 2 * ADIM], dt.bfloat16)
            nc.sync.dma_start(tw2[:], tw2d[:])
            fw1a = wpool.tile([CH, ADIM], dt.bfloat16)
            nc.sync.dma_start(fw1a[:], fw1ad[:])
            fw1b = wpool.tile([112, ADIM], dt.bfloat16)
            nc.sync.dma_start(fw1b[:], fw1bd[:])
            fw2 = wpool.tile([CH, 2, 3 * ADIM], dt.float8e4)
            nc.sync.dma_start(fw2[:], fw2d[:])
            fw3 = wpool.tile([CH, 2, ADIM], dt.float8e4)
            nc.sync.dma_start(fw3[:], fw3d[:])
            bias = wpool.tile([CH, 10], dt.float32)
            nc.sync.dma_start(bias[:], biasd[:])

            # persistent double-buffered tiles; edge cols / pad rows zeroed once
            xts_db, xf_db, h1_db, h2_db = [], [], [], []
            augl_db, augr_db, masks_db, onesr2_db = [], [], [], []
            for d_ in range(2):
                xts_ = [
                    inpool.tile([CH, T + 2], dt.bfloat16, name=f"xt{d_}{c}", tag=f"xt{d_}{c}")
                    for c in range(NCC)
                ]
                for tile_ in xts_:
                    nc.gpsimd.memset(tile_[:, 0:1], 0.0)
                    nc.gpsimd.memset(tile_[:, -1:], 0.0)
                xts_db.append(xts_)
                xfa = inpool.tile([CH, F], dt.bfloat16, name=f"xfa{d_}", tag=f"xfa{d_}")
                xfb = inpool.tile([112, F], dt.bfloat16, name=f"xfb{d_}", tag=f"xfb{d_}")
                nc.vector.memset(xfa[:], 0.0)
                nc.vector.memset(xfb[:], 0.0)
                xf_db.append((xfa, xfb))
                h1i = apool.tile([CH, NCC, F + 16], dt.float8e4, name=f"h1_{d_}", tag=f"h1_{d_}", bufs=1)
                nc.vector.memset(h1i[:], 0.0)
                h1_db.append(h1i)
                h2i_ = apool.tile([CH, NCC, F + 16], dt.float8e4, name=f"h2_{d_}", tag=f"h2_{d_}", bufs=1)
                nc.vector.memset(h2i_[:], 0.0)
                h2_db.append(h2i_)

                # aug operands padded to K=128 (rows 2.. stay zero forever)
                augl = rpool.tile([CH, F], dt.bfloat16, name=f"augl{d_}", tag=f"augl{d_}")
                nc.vector.memset(augl[:], 0.0)
                augl_db.append(augl)
                augr = rpool.tile([CH, T], dt.bfloat16, name=f"augr{d_}", tag=f"augr{d_}")
                nc.vector.memset(augr[:], 0.0)
                augr_db.append(augr)
                # masks2 row0 = ones (once), row1 = per-batch mask
                masks = rpool.tile([2, T], dt.float32, name=f"masks{d_}", tag=f"masks{d_}")
                nc.vector.memset(masks[0:1, :], 0.0)
                nc.sync.dma_start(masks[0:1, :], onesd[:1, :T])
                masks_db.append(masks)
                # [0;1] rows added to the f2 psum when building aug_l
                onesr2 = rpool.tile([2, T], dt.float32, name=f"onesr2{d_}", tag=f"onesr2{d_}")
                nc.vector.memset(onesr2[0:1, :], 0.0)
                nc.sync.dma_start(onesr2[1:2, :], onesd[:1, :T])
                onesr2_db.append(onesr2)

            # explicit ordering chain for the table-set-sensitive ACT ops
            act_chain = []

            def chain(inst):
                if act_chain:
                    add_dep_helper(inst.ins, act_chain[-1].ins, sync=False)
                act_chain.append(inst)

            # pin PE instruction order to program order so same-weight matmul
            # runs stay adjacent (their deduped Ldweights then pipeline)
            pe_chain = []

            def mm(*args, **kw):
                return nc.tensor.matmul(*args, **kw)

            for b in range(bl):
                # ---------------- text branch ----------------
                xts = xts_db[b % 2]
                for c in range(NCC):
                    nc.sync.dma_start(xts[c][:, 1 : T + 1], textt[b, c])
                ths = []
                for co in range(NCC):
                    ps = psconv.tile([CH, T], dt.float32, tag="psc", name=f"pst1_{b}_{co}")
                    idx = 0
                    for ci in range(NCC):
                        for k in range(3):
                            s = (ci * 3 + k) * ADIM + co * CH
                            mm(
                                ps[:], tw1[:, s : s + CH], xts[ci][:, k : k + T],
                                start=(idx == 0), stop=(idx == 5),
                            )
                            idx += 1
                    th = apool.tile([CH, T], dt.bfloat16, tag=f"th{co}")
                    nc.scalar.activation(th[:], ps[:], AF.Relu, bias=bias[:, co : co + 1], scale=1.0)
                    ths.append(th)
                # conv2 (k=1), weights pre-scaled by -2 -> tp = -2*t, fp8 out
                tpsi = apool.tile([CH, NCC, 512], dt.float8e4, tag="tpsi")
                for co in range(NCC):
                    ps = psconv.tile([CH, T], dt.float32, tag="psc", name=f"pst2_{b}_{co}")
                    for ci in range(NCC):
                        s = ci * ADIM + co * CH
                        mm(
                            ps[:], tw2[:, s : s + CH], ths[ci][:],
                            start=(ci == 0), stop=(ci == 1),
                        )
                    nc.scalar.activation(tpsi[:, co, :T], ps[:], AF.Identity, bias=bias[:, 2 + co : 3 + co], scale=1.0)
                # aug_r row1 = 0.25*sum tp^2 + maskinf, row0 = ones
                augr = augr_db[b % 2]
                masks = masks_db[b % 2]
                nc.sync.dma_start(masks[1:2, :], maskv[b : b + 1, :])
                pst2 = psq.tile([CH, T], dt.float32, tag="psq", name=f"psq_t2_{b}")
                for c in range(NCC):
                    tsq = sqpool.tile([CH, T], dt.bfloat16, tag="sq", name=f"tsq_{b}_{c}")
                    nc.vector.tensor_mul(tsq[:], tpsi[:, c, :T], tpsi[:, c, :T])
                    mm(pst2[:], wt2c[:], tsq[:], start=(c == 0), stop=(c == 1))
                nc.vector.tensor_add(augr[0:2, :], pst2[0:2, :], masks[0:2, :])

                # ---------------- feats branch ----------------
                xfa, xfb = xf_db[b % 2]
                nc.sync.dma_start(xfa[0:80, 1:F], featst[b][:, 0 : F - 1])
                nc.sync.dma_start(xfa[80:128, :], featst[b][0:48, :])
                nc.sync.dma_start(xfb[0:32, :], featst[b][48:80, :])
                nc.sync.dma_start(xfb[32:112, 0 : F - 1], featst[b][:, 1:F])
                h1i = h1_db[b % 2]
                for co in range(NCC):
                    pss = [psconv.tile([CH, T], dt.float32, tag="psc", name=f"psf1_{b}_{co}_{i}") for i in range(NTT)]
                    for tt in range(NTT):
                        mm(
                            pss[tt][:], fw1a[:, co * CH : co * CH + CH],
                            xfa[:, tt * T : tt * T + T], start=True, stop=False,
                        )
                    for tt in range(NTT):
                        mm(
                            pss[tt][:], fw1b[:112, co * CH : co * CH + CH],
                            xfb[:112, tt * T : tt * T + T], start=False, stop=True,
                        )
                    for tt in range(NTT):
                        nc.vector.tensor_scalar(
                            h1i[:, co, 1 + tt * T : 1 + (tt + 1) * T], pss[tt][:],
                            bias[:, 4 + co : 5 + co], 0.0, ALU.add, ALU.max,
                        )
                h2i = h2_db[b % 2]
                for co in range(NCC):
                    pss = [psconv.tile([CH, T], dt.float32, tag="psc", name=f"psf2_{b}_{co}_{i}") for i in range(NTT)]
                    for k in range(3):
                        s = k * ADIM + co * CH
                        for tt in range(NTT):
                            mm(
                                pss[tt][:], fw2[:, 0:2, s : s + CH],
                                h1i[:, 0:2, tt * T + k : tt * T + k + T],
                                start=(k == 0), stop=(k == 2),
                                perf_mode=mybir.MatmulPerfMode.DoubleRow,
                            )
                    for tt in range(NTT):
                        nc.vector.tensor_scalar(
                            h2i[:, co, 1 + tt * T : 1 + (tt + 1) * T], pss[tt][:],
                            bias[:, 6 + co : 7 + co], 0.0, ALU.add, ALU.max,
                        )
                fsi = fpool.tile([CH, NCC, F], dt.float8e4, tag="fsi")
                for co in range(NCC):
                    pss = [psconv.tile([CH, T], dt.float32, tag="psc", name=f"psf3_{b}_{co}_{i}") for i in range(NTT)]
                    for tt in range(NTT):
                        mm(
                            pss[tt][:], fw3[:, 0:2, co * CH : co * CH + CH],
                            h2i[:, 0:2, 1 + tt * T : 1 + (tt + 1) * T],
                            start=True, stop=True,
                            perf_mode=mybir.MatmulPerfMode.DoubleRow,
                        )
                    for tt in range(NTT):
                        nc.vector.tensor_scalar_add(
                            fsi[:, co, tt * T : (tt + 1) * T], pss[tt][:], bias[:, 8 + co : 9 + co]
                        )
                # aug_l row0 = sum_c f^2, row1 = ones
                augl = augl_db[b % 2]
                onesr2 = onesr2_db[b % 2]
                for tt in range(NTT):
                    psf2 = psq.tile([CH, T], dt.float32, tag="psq", name=f"psq_f2_{b}_{tt}")
                    for c in range(NCC):
                        fsq = sqpool.tile([CH, T], dt.bfloat16, tag="sq", name=f"fsq_{b}_{tt}_{c}")
                        nc.vector.tensor_mul(
                            fsq[:], fsi[:, c, tt * T : (tt + 1) * T], fsi[:, c, tt * T : (tt + 1) * T]
                        )
                        mm(psf2[:], wf2c[:], fsq[:], start=(c == 0), stop=(c == 1))
                    nc.vector.tensor_add(
                        augl[0:2, tt * T : (tt + 1) * T], psf2[0:2, :], onesr2[0:2, :]
                    )

                # ---------------- distance + log-softmax ----------------
                s_all = spool.tile([CH, len(ICHUNKS)], dt.float32, tag="sall")
                nc.vector.memset(s_all[:], 1.0)
                dists = []
                for idx, (i0, m) in enumerate(ICHUNKS):
                    ps = psft.tile([CH, T], dt.float32, tag="psf", name=f"psft_{b}_{idx}")
                    mm(
                        ps[:m], fsi[:, 0:2, i0 : i0 + m], tpsi[:, 0:2, :T],
                        start=True, stop=False,
                        perf_mode=mybir.MatmulPerfMode.DoubleRow,
                    )
                    mm(ps[:m], augl[:, i0 : i0 + m], augr[:, :T], start=False, stop=True)
                    d_ = dpool.tile([CH, T], dt.float32, tag="dist", name=f"dist_{b}_{idx}")
                    inst = nc.scalar.activation(d_[:m], ps[:m], AF.Sqrt)
                    chain(inst)
                    dists.append((d_, i0, m, idx))
                logs = spool.tile([CH, len(ICHUNKS)], dt.float32, tag="logs")
                QL = 4  # ln after every QL exps -> finals can start early
                for d_, i0, m, idx in dists:
                    e_ = epool.tile([CH, T], dt.bfloat16, tag="e", name=f"e_{b}_{idx}")
                    inst = nc.scalar.activation(
                        e_[:m], d_[:m], AF.Exp, scale=-1.0,
                        accum_out=s_all[:m, idx : idx + 1],
                    )
                    chain(inst)
                    if idx % QL == QL - 1:
                        inst = nc.scalar.activation(
                            logs[:, idx - QL + 1 : idx + 1],
                            s_all[:, idx - QL + 1 : idx + 1], AF.Ln,
                        )
                        chain(inst)
                for d_, i0, m, idx in dists:
                    o_ = opool.tile([CH, T], dt.float32, tag="o", name=f"o_{b}_{idx}")
                    nc.vector.tensor_scalar(
                        o_[:m], d_[:m], logs[:m, idx : idx + 1], -1.0, ALU.add, ALU.mult
                    )
                    nc.sync.dma_start(outd[b, i0 : i0 + m, :], o_[:m])

    _dedupe_ldweights(nc)
    nc.compile()
    _retarget_act_table_loads(nc)
    return nc


def _get_nc(bl=BL):
    key = ("nc", bl)
    if key not in _CACHE:
        _CACHE[key] = _build(bl)
    return _CACHE[key]


def _prep(inputs):
    text = np.asarray(inputs["text"], np.float32)
    feats = np.asarray(inputs["feats"], np.float32)
    xm = np.asarray(inputs["x_masks"]).astype(bool)
    tW1 = np.asarray(inputs["tW1"], np.float32)
    tb1 = np.asarray(inputs["tb1"], np.float32)
    tW2 = np.asarray(inputs["tW2"], np.float32)
    tb2 = np.asarray(inputs["tb2"], np.float32)
    fW1 = np.asarray(inputs["fW1"], np.float32)
    fb1 = np.asarray(inputs["fb1"], np.float32)
    fW2 = np.asarray(inputs["fW2"], np.float32)
    fb2 = np.asarray(inputs["fb2"], np.float32)
    fW3 = np.asarray(inputs["fW3"], np.float32)
    fb3 = np.asarray(inputs["fb3"], np.float32)

    textt = np.ascontiguousarray(text.transpose(0, 2, 1)).reshape(B, NCC, CH, T).astype(BF16)
    featst = np.ascontiguousarray(feats.transpose(0, 2, 1)).astype(BF16)
    maskv = np.where(xm, np.inf, 0.0).astype(np.float32)

    def pack_k3(W):  # (co, 256, 3) -> (128, [cc][k][co])
        t = W.transpose(1, 2, 0).reshape(NCC, CH, 3, ADIM)
        return np.ascontiguousarray(t.transpose(1, 0, 2, 3).reshape(CH, 6 * ADIM)).astype(BF16)

    def pack_k1(W):  # (co, 256) -> (128, [cc][co])
        t = W.T.reshape(NCC, CH, ADIM)
        return np.ascontiguousarray(t.transpose(1, 0, 2).reshape(CH, 2 * ADIM)).astype(BF16)

    E4 = ml_dtypes.float8_e4m3
    tw1 = pack_k3(tW1)
    tw2 = pack_k1(-2.0 * tW2[:, :, 0])
    # im2col conv1f weights: rows = (k,ci) stacked [k0:80 | k1:48][k1:32 | k2:80]
    wk = fW1.transpose(2, 1, 0)  # (k, ci, co)
    stack = np.concatenate([wk[0], wk[1], wk[2]], axis=0)  # (240, co)
    fw1a = np.ascontiguousarray(stack[0:128]).astype(BF16)
    fw1b = np.ascontiguousarray(stack[128:240]).astype(BF16)
    # conv2f DoubleRow fp8 weights: [p, ci_half, k*256+co]
    t2_ = fW2.transpose(1, 2, 0).reshape(NCC, CH, 3, ADIM)  # (half, p, k, co)
    fw2 = np.ascontiguousarray(t2_.transpose(1, 0, 2, 3).reshape(CH, NCC, 3 * ADIM)).astype(E4)
    t3_ = fW3[:, :, 0].T.reshape(NCC, CH, ADIM)  # (half, p, co)
    fw3 = np.ascontiguousarray(t3_.transpose(1, 0, 2)).astype(E4)
    bias = np.zeros((CH, 10), np.float32)
    bias[:, 0:2] = tb1.reshape(NCC, CH).T
    bias[:, 2:4] = (-2.0 * tb2).reshape(NCC, CH).T
    bias[:, 4:6] = fb1.reshape(NCC, CH).T
    bias[:, 6:8] = fb2.reshape(NCC, CH).T
    bias[:, 8:10] = fb3.reshape(NCC, CH).T

    shared = {
        "tw1": tw1, "tw2": tw2, "fw1a": fw1a, "fw1b": fw1b, "fw2": fw2, "fw3": fw3, "biases": bias,
        "onesd": np.ones((1, 512), np.float32),
    }
    in_maps = []
    for i in range(NCORES):
        m = dict(shared)
        m["textt"] = textt[i * BL : (i + 1) * BL]
        m["featst"] = featst[i * BL : (i + 1) * BL]
        m["maskv"] = maskv[i * BL : (i + 1) * BL]
        in_maps.append(m)
    return in_maps


def run(inputs, trace=False):
    from concourse.bass_utils import run_bass_kernel_spmd

    nc = _get_nc()
    in_maps = _prep(inputs)
    res = run_bass_kernel_spmd(nc, in_maps, core_ids=list(range(NCORES)), trace=trace)
    out = np.concatenate([res.results[i]["out"] for i in range(NCORES)], axis=0)
    return out, res


def kernel(**inputs):
    out, _ = run(inputs, trace=False)
    return out
